# revision 1
# baseline (speedup 1.0000x reference)
"""Distributed Bass kernel for AttnLinearEncoder (GAT-style attention encoder).

Math (reference):
    w = g * v / ||v||_row                      # weight-norm linear  [F, D]
    z = x @ w.T + b                            # [N, F]
    s = z @ a_src ; d = z @ a_dst              # [N]
    e[i, j] = relu(s_i + d_j)                  # never materialized here
    attention = softmax(e, axis=1)
    out = softmax(attention @ z + z, axis=-1)  # [N, F]

Key identity: exp(relu(u)) = max(exp(u), 1) (exp is monotonic), so the
softmax numerator P[i,j] = max(exp(s_i) * exp(d_j), 1) is a rank-1 outer
product clamped at 1 -- no transcendentals in the O(N^2) inner loop, just
one fused multiply+max per tile on the vector engine, feeding bf16 matmuls
that accumulate both attention@z and the softmax denominator (via a ones
column carried next to z in the gathered buffer).

Sharding: rows of x are striped across 8 cores (N/8 = 1536 rows each).
Each core computes its z stripe + d stripe, AllGathers
[z_bf16 | ones_bf16 | d_f32(as 2 bf16 slots)] (N x 131 bf16) in two
halves (attention on half 1 overlaps the gather of half 2), then
computes its 1536 x N attention stripe against the full z.
"""

import numpy as np
from contextlib import ExitStack

import concourse.bass as bass
import concourse.bacc as bacc
import concourse.mybir as mybir
import concourse.tile as tile
from concourse.bass_utils import run_bass_kernel_spmd

FP32 = mybir.dt.float32
BF16 = mybir.dt.bfloat16

N_TOTAL = 12288
D = 512
F = 128
NCORES = 8
P = 128
RW = F + 2          # gathered z-row width: z(128) | ones | pad
BW = RW + 2         # rank block bf16 elems per row incl. d region


def build(n_total=N_TOTAL, ncores=NCORES, timing_reps=0, tlsim=False):
    stripe = n_total // ncores          # rows per core
    nib = stripe // P                   # i-blocks of 128 own rows
    njt = n_total // P                  # j-tiles of 128 global rows
    nkc = D // P                        # k-chunks of the input dim
    nbw = min(512, stripe)              # moving free dim per z matmul
    nnb = stripe // nbw
    assert nib % 2 == 0
    nibh = nib // 2                     # i-blocks per gather half
    hst = stripe // 2                   # rows per gather half

    nc = bacc.Bacc("TRN2", target_bir_lowering=False, debug=False,
                   num_devices=1 if tlsim else ncores)

    xT = nc.dram_tensor("xT", [D, stripe], FP32, kind="ExternalInput")
    v_ext = nc.dram_tensor("v", [F, D], FP32, kind="ExternalInput")
    vT_ext = nc.dram_tensor("vT", [D, F], FP32, kind="ExternalInput")
    g_ext = nc.dram_tensor("g", [F, 1], FP32, kind="ExternalInput")
    b_ext = nc.dram_tensor("b", [F, 1], FP32, kind="ExternalInput")
    aw_ext = nc.dram_tensor("aw", [2 * F, 1], FP32, kind="ExternalInput")
    id_ext = nc.dram_tensor("id128", [P, P], FP32, kind="ExternalInput")
    out_ext = nc.dram_tensor("out", [stripe, F], FP32, kind="ExternalOutput")

    with tile.TileContext(nc) as tc, ExitStack() as ctx:
        const = ctx.enter_context(tc.tile_pool(name="const", bufs=1))
        dram = ctx.enter_context(tc.tile_pool(name="dram", bufs=1, space="DRAM"))
        psum_ctx = ExitStack()
        psum = psum_ctx.enter_context(
            tc.tile_pool(name="psum", bufs=2, space="PSUM"))
        work = ctx.enter_context(tc.tile_pool(name="work", bufs=1))

        def rep_loop():
            if timing_reps <= 0:
                return None
            cm = tc.For_i(0, timing_reps, 1,
                          hint_engines=(mybir.EngineType.PE,
                                        mybir.EngineType.DVE,
                                        mybir.EngineType.Activation,
                                        mybir.EngineType.SP))
            cm.__enter__()
            return cm

        def ptile(shape):
            # transient PSUM tiles share the "tmp" tag -> 2 rotating slots
            return psum.tile(shape, FP32, tag="tmp", name="ptmp")

        # rank block layout (bf16 elems): [hst x RW z|1 rows][2*hst d-f32]
        zc_loc = [dram.tile([hst * BW], BF16, name=f"zc_loc{h}") for h in (0, 1)]
        zc_full = [dram.tile([ncores * hst * BW], BF16, addr_space="Shared",
                             name=f"zc_full{h}") for h in (0, 1)]

        def blk_z(buf, base):      # [hst, RW] z|1 rows of one rank block
            return buf[base:base + hst * RW].rearrange("(i w) -> i w", w=RW)

        def blk_d(buf, base):      # [hst] f32 d region of one rank block
            return buf[base + hst * RW:base + hst * BW].bitcast(FP32)

        # ---- constants -------------------------------------------------
        v_sb = const.tile([P, D], FP32)
        vT_sb = const.tile([P, nkc, F], FP32)
        g_sb = const.tile([P, 1], FP32)
        b_sb = const.tile([P, 1], FP32)
        asad = const.tile([P, 2], FP32)
        ident = const.tile([P, P], FP32)
        ones_row = const.tile([1, P], FP32)
        nc.vector.memset(ones_row[:], 1.0)
        nc.gpsimd.dma_start(v_sb[:], v_ext[:])
        nc.gpsimd.dma_start(vT_sb[:], vT_ext.ap().rearrange("(c p) f -> p c f", p=P))
        nc.gpsimd.dma_start(g_sb[:], g_ext[:])
        nc.gpsimd.dma_start(b_sb[:], b_ext[:])
        nc.gpsimd.dma_start(asad[:, 0:1], aw_ext[0:F, :])
        nc.gpsimd.dma_start(asad[:, 1:2], aw_ext[F:2 * F, :])
        nc.gpsimd.dma_start(ident[:], id_ext[:])

        xc = [work.tile([P, stripe], FP32, name=f"xc{c}") for c in range(nkc)]
        rep_a = rep_loop()
        xT_v = xT.ap().rearrange("(c p) i -> c p i", p=P)
        for c in range(nkc):
            nc.sync.dma_start(xc[c][:], xT_v[c])

        # ---- weight prep: scale = g / ||v||_row ------------------------
        # The scale never touches the weights: z = (x @ v.T) * scale + b is
        # applied per-partition at the PSUM eviction, so the z matmuls start
        # as soon as vT and the first x chunk land.
        v2 = work.tile([P, D], FP32)
        nc.vector.tensor_mul(v2[:], v_sb[:], v_sb[:])
        nrm2 = work.tile([P, 1], FP32)
        nc.vector.reduce_sum(nrm2[:], v2[:], axis=mybir.AxisListType.X)
        nrm = work.tile([P, 1], FP32)
        nc.scalar.sqrt(nrm[:], nrm2[:])
        rinv = work.tile([P, 1], FP32)
        nc.vector.reciprocal(rinv[:], nrm[:])
        scale_w = work.tile([P, 1], FP32)
        nc.vector.tensor_mul(scale_w[:], rinv[:], g_sb[:])

        # ---- z stripe (transposed) + s/d from zT -----------------------
        # s = z @ a_src, d = z @ a_dst (scale/bias already folded into z)
        zT_sb = work.tile([P, stripe], FP32)
        sd_sb = work.tile([2, stripe], FP32)
        for nb in range(nnb):
            sl = slice(nb * nbw, (nb + 1) * nbw)
            zt_ps = ptile([P, nbw])
            for c in range(nkc):
                nc.tensor.matmul(zt_ps[:], vT_sb[:, c, :], xc[c][:, sl],
                                 start=(c == 0), stop=(c == nkc - 1))
            nc.scalar.activation(zT_sb[:, sl], zt_ps[:],
                                 mybir.ActivationFunctionType.Identity,
                                 bias=b_sb[:], scale=scale_w[:])
            sd_ps = ptile([2, nbw])
            nc.tensor.matmul(sd_ps[:], asad[:], zT_sb[:, sl],
                             start=True, stop=True)
            nc.scalar.copy(sd_sb[:, sl], sd_ps[:])

        # z natural layout: f32 for +z / output, bf16 (+ones col) for gather
        zn_sb = work.tile([P, nib, F], FP32)
        znb_sb = work.tile([P, nib, RW], BF16)
        nc.vector.memset(znb_sb[:, :, F:RW], 1.0)
        for ib in range(nib):
            zn_ps = ptile([P, P])
            nc.tensor.transpose(zn_ps[:], zT_sb[:, ib * P:(ib + 1) * P], ident[:])
            nc.scalar.copy(zn_sb[:, ib, :], zn_ps[:])
            nc.vector.tensor_copy(znb_sb[:, ib, 0:F], zn_sb[:, ib, :])
            h, lb = divmod(ib, nibh)
            nc.sync.dma_start(blk_z(zc_loc[h], 0)[lb * P:(lb + 1) * P, :],
                              znb_sb[:, ib, :])
        for h in (0, 1):
            nc.sync.dma_start(blk_d(zc_loc[h], 0), sd_sb[1:2, h * hst:(h + 1) * hst])

        # Es[i] = exp(s_i) broadcast over partitions, bf16 [128, stripe]
        # (depends only on local sd, so it runs under the all-gather)
        es_bc = work.tile([P, stripe], BF16)
        for nb in range(nnb):
            sl = slice(nb * nbw, (nb + 1) * nbw)
            es_ps = ptile([P, nbw])
            nc.tensor.matmul(es_ps[:], ones_row[:], sd_sb[0:1, sl],
                             start=True, stop=True)
            nc.scalar.activation(es_bc[:, sl], es_ps[:],
                                 mybir.ActivationFunctionType.Exp)

        if rep_a is not None:
            rep_a.__exit__(None, None, None)

        # ---- all-gather [z | 1 | d], two halves ------------------------
        for h in (0, 1):
            if tlsim:
                nc.gpsimd.dma_start(zc_full[h][0:hst * BW], zc_loc[h][:])
            else:
                nc.gpsimd.collective_compute(
                    "AllGather",
                    mybir.AluOpType.bypass,
                    ins=[zc_loc[h][:].opt()],
                    outs=[zc_full[h][:].opt()],
                    replica_groups=[list(range(ncores))],
                )

        # j-tile t -> (half, row block) in the gathered buffers
        def t_loc(t):
            r, l = divmod(t, nib)
            h, lb = divmod(l, nibh)
            return h, (r * nibh + lb)

        torder = sorted(range(njt), key=lambda t: t_loc(t))

        njth = njt // 2
        rep_b = rep_loop()
        # ---- post-gather prep -----------------------------------------
        # Ed[j] = exp(d_j) as per-partition columns [128, njt] in gather
        # order; one contiguous DMA per (half, rank)
        ed_h = [work.tile([P, njth], FP32, name=f"ed{h}") for h in (0, 1)]
        for h in (0, 1):
            for r in range(ncores):
                src = (blk_d(zc_full[h], r * hst * BW)
                       .rearrange("(l p) -> p l", p=P))
                nc.sync.dma_start(ed_h[h][:, r * nibh:(r + 1) * nibh], src)
            nc.scalar.activation(ed_h[h][:], ed_h[h][:],
                                 mybir.ActivationFunctionType.Exp)

        # gathered z|1 rows land in SBUF in gather order, one tile+DMA per
        # (half, rank) so the attention can start after the first block;
        # attention reads [z | 1] slices (cols 0:129)
        rhs_hr = [work.tile([P, nibh, RW], BF16, name=f"rhs{h}_{r}")
                  for h in (0, 1) for r in range(ncores)]
        for h in (0, 1):
            for r in range(ncores):
                nc.sync.dma_start(
                    rhs_hr[h * ncores + r][:],
                    blk_z(zc_full[h], r * hst * BW)
                    .rearrange("(q p) w -> p q w", p=P))

        # ---- attention stripe: accumulate P.T @ [z|1] over all j ------
        # One PSUM bank per i-block accumulator; the tmp psum pool is
        # closed here so all 8 banks are available: passes of 8 then 4
        # (shorter final epilogue tail).
        psum_ctx.close()
        apsum = ctx.enter_context(tc.tile_pool(name="apsum", bufs=1, space="PSUM"))
        ptp = ctx.enter_context(tc.tile_pool(name="ptp", bufs=4))
        epi = ctx.enter_context(tc.tile_pool(name="epi", bufs=4))
        ib_group = 8
        for ib0 in range(0, nib, ib_group):
            ngrp = min(ib_group, nib - ib0)
            gw = ngrp * P
            accs = [apsum.tile([P, F + 1], FP32, name=f"acc{a}", tag=f"acc{a}")
                    for a in range(ngrp)]
            for ti, t in enumerate(torder):
                pt = ptp.tile([P, gw], BF16, tag="pt", name="pt")
                nc.vector.tensor_scalar(pt[:], es_bc[:, ib0 * P:ib0 * P + gw],
                                        ed_h[ti // njth][:, ti % njth:ti % njth + 1],
                                        1.0,
                                        op0=mybir.AluOpType.mult,
                                        op1=mybir.AluOpType.max)
                rhs_t = rhs_hr[ti // nibh][:, ti % nibh, 0:F + 1]
                for a in range(ngrp):
                    nc.tensor.matmul(accs[a][:],
                                     pt[:, a * P:(a + 1) * P],
                                     rhs_t,
                                     start=(ti == 0), stop=(ti == njt - 1))

            # epilogue: attn = num/den, z2 = attn + z, softmax over F.
            # z2 is in [-14, 14] so exp is f32-safe without max-subtraction.
            # Per-bank scalar ops only where the per-block denominator
            # forces it; everything else is one wide op per pass.
            z2w = epi.tile([P, ngrp, F], FP32, tag="z2w", name="z2w")
            for a in range(ngrp):
                acc = accs[a][:]
                rden = epi.tile([P, 1], FP32, tag=f"rden{a}", name="rden")
                nc.vector.reciprocal(rden[:], acc[:, F:F + 1])
                # PSUM->SBUF stage fused with the 1/den scale; frees the bank
                nc.scalar.mul(z2w[:, a, :], acc[:, 0:F], rden[:])
            nc.vector.tensor_add(z2w[:], z2w[:], zn_sb[:, ib0:ib0 + ngrp, :])
            e2w = epi.tile([P, ngrp, F], FP32, tag="e2w", name="e2w")
            nc.scalar.activation(e2w[:], z2w[:],
                                 mybir.ActivationFunctionType.Exp)
            s6 = epi.tile([P, ngrp], FP32, tag="s6", name="s6")
            nc.vector.reduce_sum(s6[:], e2w[:], axis=mybir.AxisListType.X)
            r6 = epi.tile([P, ngrp], FP32, tag="r6", name="r6")
            nc.vector.reciprocal(r6[:], s6[:])
            o_w = epi.tile([P, ngrp, F], FP32, tag="o_w", name="o_w")
            for a in range(ngrp):
                nc.vector.tensor_scalar_mul(o_w[:, a, :], e2w[:, a, :],
                                            r6[:, a:a + 1])
            nc.sync.dma_start(
                out_ext[ib0 * P:(ib0 + ngrp) * P, :]
                .rearrange("(a p) f -> p a f", p=P),
                o_w[:])

        if rep_b is not None:
            rep_b.__exit__(None, None, None)

    nc.compile()
    return nc


_CACHE = {}


def _get_nc(n_total=N_TOTAL, ncores=NCORES):
    key = (n_total, ncores)
    if key not in _CACHE:
        _CACHE[key] = build(n_total, ncores)
    return _CACHE[key]


def make_in_maps(x, v, g, b, att_weights, ncores=NCORES):
    n_total = x.shape[0]
    stripe = n_total // ncores
    x = np.ascontiguousarray(np.asarray(x, np.float32))
    xT = np.ascontiguousarray(x.T)
    v = np.ascontiguousarray(np.asarray(v, np.float32))
    vT = np.ascontiguousarray(v.T)
    g = np.ascontiguousarray(np.asarray(g, np.float32).reshape(F, 1))
    b = np.ascontiguousarray(np.asarray(b, np.float32).reshape(F, 1))
    aw = np.ascontiguousarray(np.asarray(att_weights, np.float32).reshape(2 * F, 1))
    id128 = np.eye(P, dtype=np.float32)
    maps = []
    for c in range(ncores):
        maps.append({
            "xT": np.ascontiguousarray(xT[:, c * stripe:(c + 1) * stripe]),
            "v": v, "vT": vT, "g": g, "b": b, "aw": aw, "id128": id128,
        })
    return maps


def kernel(x, v, g, b, att_weights):
    n_total = x.shape[0]
    nc = _get_nc(n_total, NCORES)
    in_maps = make_in_maps(x, v, g, b, att_weights, NCORES)
    res = run_bass_kernel_spmd(nc, in_maps, core_ids=list(range(NCORES)))
    out = np.concatenate([res.results[c]["out"] for c in range(NCORES)], axis=0)
    return out.astype(np.float32)



# revision 15
# speedup vs baseline: 1.0827x; 1.0827x over previous
"""Distributed Bass kernel for AttnLinearEncoder (GAT-style attention encoder).

Math (reference):
    w = g * v / ||v||_row                      # weight-norm linear  [F, D]
    z = x @ w.T + b                            # [N, F]
    s = z @ a_src ; d = z @ a_dst              # [N]
    e[i, j] = relu(s_i + d_j)                  # never materialized here
    attention = softmax(e, axis=1)
    out = softmax(attention @ z + z, axis=-1)  # [N, F]

Key identity: exp(relu(u)) = max(exp(u), 1) (exp is monotonic), so the
softmax numerator P[i,j] = max(exp(s_i) * exp(d_j), 1) is a rank-1 outer
product clamped at 1 -- no transcendentals in the O(N^2) inner loop, just
one fused multiply+max per tile on the vector engine (bf16 in/out, so the
DVE runs in its 4x perf mode), feeding bf16 matmuls that accumulate both
attention@z and the softmax denominator via a ones column carried next to
z in the gathered buffer.

Sharding: rows of x are striped across 8 cores (N/8 = 1536 rows each).
Each core computes its z stripe + d stripe in bf16 (x is pre-cast to bf16
on the host; the z matmuls run at the PE's 1-cycle/row bf16 rate instead
of fp32's 4), AllGathers rank blocks of [d_f32 | z_bf16 x128 | 1 | pad]
rows (N x 132 bf16) in two halves (attention on half 1 overlaps the
gather of half 2), then computes its 1536 x N attention stripe against
the full z. d rides at the front of each row, 4-byte aligned, so rep_b
needs no separate d gather pass -- each [128,6,132] rhs tile carries its
own d column, exp'd by a tiny per-tile Act op.
"""

import numpy as np
import ml_dtypes
from contextlib import ExitStack

import concourse.bass as bass
import concourse.bacc as bacc
import concourse.mybir as mybir
import concourse.tile as tile
from concourse.bass_utils import run_bass_kernel_spmd

FP32 = mybir.dt.float32
BF16 = mybir.dt.bfloat16

N_TOTAL = 12288
D = 512
F = 128
NCORES = 8
P = 128
BW = 132            # row: d_f32(2 slots) | z(128) | ones | pad
ZOFF = 2            # z starts at slot 2; [z|1] = slots 2:131


def build(n_total=N_TOTAL, ncores=NCORES, timing_reps=0, tlsim=False):
    stripe = n_total // ncores          # rows per core
    nib = stripe // P                   # i-blocks of 128 own rows
    njt = n_total // P                  # j-tiles of 128 global rows
    nkc = D // P                        # k-chunks of the input dim
    nbw = min(512, stripe)              # moving free dim per z matmul
    nnb = stripe // nbw
    assert nib % 2 == 0
    nibh = nib // 2                     # i-blocks per gather half
    hst = stripe // 2                   # rows per gather half

    nc = bacc.Bacc("TRN2", target_bir_lowering=False, debug=False,
                   num_devices=1 if tlsim else ncores)

    xT = nc.dram_tensor("xT", [D, stripe], BF16, kind="ExternalInput")
    v_ext = nc.dram_tensor("v", [F, D], FP32, kind="ExternalInput")
    vT_ext = nc.dram_tensor("vT", [D, F], BF16, kind="ExternalInput")
    g_ext = nc.dram_tensor("g", [F, 1], FP32, kind="ExternalInput")
    b_ext = nc.dram_tensor("b", [F, 1], FP32, kind="ExternalInput")
    aw_ext = nc.dram_tensor("aw", [2 * F, 1], FP32, kind="ExternalInput")
    id_ext = nc.dram_tensor("id128", [P, P], FP32, kind="ExternalInput")
    out_ext = nc.dram_tensor("out", [stripe, F], FP32, kind="ExternalOutput")

    with tile.TileContext(nc) as tc, ExitStack() as ctx:
        const = ctx.enter_context(tc.tile_pool(name="const", bufs=1))
        dram = ctx.enter_context(tc.tile_pool(name="dram", bufs=1, space="DRAM"))
        psum_ctx = ExitStack()
        psum = psum_ctx.enter_context(
            tc.tile_pool(name="psum", bufs=2, space="PSUM"))
        work = ctx.enter_context(tc.tile_pool(name="work", bufs=1))

        def rep_loop():
            if timing_reps <= 0:
                return None
            cm = tc.For_i(0, timing_reps, 1,
                          hint_engines=(mybir.EngineType.PE,
                                        mybir.EngineType.DVE,
                                        mybir.EngineType.Activation,
                                        mybir.EngineType.SP))
            cm.__enter__()
            return cm

        def ptile(shape, dt=FP32):
            # transient PSUM tiles share the "tmp" tag -> 2 rotating slots
            return psum.tile(shape, dt, tag="tmp", name="ptmp")

        # rank block layout (bf16 elems): hst rows of BW = [d|z|1|pad]
        zc_loc = [dram.tile([hst * BW], BF16, name=f"zc_loc{h}") for h in (0, 1)]
        zc_full = [dram.tile([ncores * hst * BW], BF16, addr_space="Shared",
                             name=f"zc_full{h}") for h in (0, 1)]

        def blk(buf, base):        # [hst, BW] rows of one rank block
            return buf[base:base + hst * BW].rearrange("(i w) -> i w", w=BW)

        # ---- constants -------------------------------------------------
        v_sb = const.tile([P, D], FP32)
        vT_sb = const.tile([P, nkc, F], BF16)
        g_sb = const.tile([P, 1], FP32)
        b_sb = const.tile([P, 1], FP32)
        asad = const.tile([P, 2], BF16)
        asad32 = const.tile([P, 2], FP32)
        ident = const.tile([P, P], FP32)
        ones_row = const.tile([1, P], BF16)
        nc.vector.memset(ones_row[:], 1.0)
        nc.gpsimd.dma_start(v_sb[:], v_ext[:])
        nc.gpsimd.dma_start(vT_sb[:], vT_ext.ap().rearrange("(c p) f -> p c f", p=P))
        nc.gpsimd.dma_start(g_sb[:], g_ext[:])
        nc.gpsimd.dma_start(b_sb[:], b_ext[:])
        nc.gpsimd.dma_start(asad32[:, 0:1], aw_ext[0:F, :])
        nc.gpsimd.dma_start(asad32[:, 1:2], aw_ext[F:2 * F, :])
        nc.gpsimd.dma_start(ident[:], id_ext[:])
        nc.vector.tensor_copy(asad[:], asad32[:])

        xc = [work.tile([P, stripe], BF16, name=f"xc{c}") for c in range(nkc)]
        rep_a = rep_loop()
        xT_v = xT.ap().rearrange("(c p) i -> c p i", p=P)
        for c in range(nkc):
            # split the input stream across both HWDGE queues
            eng = nc.sync if c % 2 == 0 else nc.scalar
            eng.dma_start(xc[c][:], xT_v[c])

        # ---- weight prep: scale = g / ||v||_row ------------------------
        # The scale never touches the weights: z = (x @ v.T) * scale + b is
        # applied per-partition at the PSUM eviction, so the z matmuls start
        # as soon as vT and the first x chunk land.
        v2 = work.tile([P, D], FP32)
        nc.vector.tensor_mul(v2[:], v_sb[:], v_sb[:])
        nrm2 = work.tile([P, 1], FP32)
        nc.vector.reduce_sum(nrm2[:], v2[:], axis=mybir.AxisListType.X)
        nrm = work.tile([P, 1], FP32)
        nc.scalar.sqrt(nrm[:], nrm2[:])
        rinv = work.tile([P, 1], FP32)
        nc.vector.reciprocal(rinv[:], nrm[:])
        scale_w = work.tile([P, 1], FP32)
        nc.vector.tensor_mul(scale_w[:], rinv[:], g_sb[:])

        # ---- z stripe (transposed) + exp(s)/exp(d) from zT -------------
        # s = z @ a_src, d = z @ a_dst (scale/bias already folded into z)
        zT_sb = work.tile([P, stripe], FP32)
        zTb_sb = work.tile([P, stripe], BF16)
        esd_sb = work.tile([2, stripe], FP32)
        esb_sb = work.tile([1, stripe], BF16)
        for nb in range(nnb):
            sl = slice(nb * nbw, (nb + 1) * nbw)
            zt_ps = ptile([P, nbw])
            for c in range(nkc):
                nc.tensor.matmul(zt_ps[:], vT_sb[:, c, :], xc[c][:, sl],
                                 start=(c == 0), stop=(c == nkc - 1))
            nc.scalar.activation(zT_sb[:, sl], zt_ps[:],
                                 mybir.ActivationFunctionType.Identity,
                                 bias=b_sb[:], scale=scale_w[:])
            nc.vector.tensor_copy(zTb_sb[:, sl], zT_sb[:, sl])
            sd_ps = ptile([2, nbw])
            nc.tensor.matmul(sd_ps[:], asad[:], zTb_sb[:, sl],
                             start=True, stop=True)
            # exp both rows straight out of PSUM: [exp(s); exp(d)]
            nc.scalar.activation(esd_sb[:, sl], sd_ps[:],
                                 mybir.ActivationFunctionType.Exp)
            nc.vector.tensor_copy(esb_sb[:, sl], esd_sb[0:1, sl])

        # z natural layout: f32 for +z / output, bf16 (+ones col) for gather
        zn_sb = work.tile([P, nib, F], FP32)
        znb_sb = work.tile([P, nib, F + 1], BF16)
        nc.vector.memset(znb_sb[:, :, F:F + 1], 1.0)
        for ib in range(nib):
            zn_ps = ptile([P, P])
            nc.tensor.transpose(zn_ps[:], zT_sb[:, ib * P:(ib + 1) * P], ident[:])
            nc.scalar.copy(zn_sb[:, ib, :], zn_ps[:])
            nc.vector.tensor_copy(znb_sb[:, ib, 0:F], zn_sb[:, ib, :])
        for h in (0, 1):
            # one [z|1] row-block write per half, one d column write per half
            eng = nc.sync if h == 0 else nc.scalar
            eng.dma_start(
                blk(zc_loc[h], 0)[:, ZOFF:ZOFF + F + 1]
                .rearrange("(q p) w -> p q w", p=P),
                znb_sb[:, h * nibh:(h + 1) * nibh, :])
            # exp(d) rides at the front of each row, 4-byte aligned
            eng.dma_start(
                blk(zc_loc[h], 0)[:, 0:2].bitcast(FP32),
                esd_sb[1:2, h * hst:(h + 1) * hst])

        # Es[i] = exp(s_i) broadcast over partitions, bf16 [128, stripe]
        # (depends only on local sd, so it runs under the all-gather)
        es_bc = work.tile([P, stripe], BF16)
        for nb in range(nnb):
            sl = slice(nb * nbw, (nb + 1) * nbw)
            es_ps = ptile([P, nbw])
            nc.tensor.matmul(es_ps[:], ones_row[:], esb_sb[:, sl],
                             start=True, stop=True)
            nc.vector.tensor_copy(es_bc[:, sl], es_ps[:])

        if rep_a is not None:
            rep_a.__exit__(None, None, None)

        # ---- all-gather [d | z | 1], two halves ------------------------
        for h in (0, 1):
            if tlsim:
                nc.gpsimd.dma_start(zc_full[h][0:hst * BW], zc_loc[h][:])
            else:
                nc.gpsimd.collective_compute(
                    "AllGather",
                    mybir.AluOpType.bypass,
                    ins=[zc_loc[h][:].opt()],
                    outs=[zc_full[h][:].opt()],
                    replica_groups=[list(range(ncores))],
                )

        # j-tile t -> (half, row block) in the gathered buffers
        def t_loc(t):
            r, l = divmod(t, nib)
            h, lb = divmod(l, nibh)
            return h, (r * nibh + lb)

        torder = sorted(range(njt), key=lambda t: t_loc(t))

        rep_b = rep_loop()
        # ---- post-gather prep -----------------------------------------
        # gathered rows land in SBUF in gather order, one tile+DMA per
        # (half, rank-pair) so the attention can start after the first
        # block; each tile carries exp(d)(f32) | z | 1 per row, so there
        # is NO prep compute at all -- the pt build reads its per-row
        # exp(d) scalar straight out of the rhs tile via a strided bitcast
        # slice. (DMA issues cost ~0.7us of sequencer time each, so fewer,
        # larger transfers win.)
        rpb = 2                          # ranks per rhs block
        nrb = ncores // rpb              # rhs blocks per half
        bq = rpb * nibh                  # j-tiles per rhs block
        rhs_hr = [work.tile([P, bq, BW], BF16, name=f"rhs{h}_{r}")
                  for h in (0, 1) for r in range(nrb)]
        for h in (0, 1):
            for r in range(nrb):
                eng = nc.sync if r % 2 == 0 else nc.scalar
                base = r * rpb * hst * BW
                src = (zc_full[h][base:base + rpb * hst * BW]
                       .rearrange("(q p w) -> p q w", p=P, w=BW))
                eng.dma_start(rhs_hr[h * nrb + r][:], src)

        # ---- attention stripe: accumulate P.T @ [z|1] over all j ------
        # One PSUM bank per i-block accumulator; the tmp psum pool is
        # closed here so all 8 banks are available: passes of 8 then 4
        # (shorter final epilogue tail).
        psum_ctx.close()
        apsum = ctx.enter_context(tc.tile_pool(name="apsum", bufs=1, space="PSUM"))
        ptp = ctx.enter_context(tc.tile_pool(name="ptp", bufs=4))
        epi = ctx.enter_context(tc.tile_pool(name="epi", bufs=4))
        ib_group = 8
        for ib0 in range(0, nib, ib_group):
            ngrp = min(ib_group, nib - ib0)
            gw = ngrp * P
            accs = [apsum.tile([P, F + 1], FP32, name=f"acc{a}", tag=f"acc{a}")
                    for a in range(ngrp)]
            for ti, t in enumerate(torder):
                blkno, l = divmod(ti, bq)
                pt = ptp.tile([P, gw], BF16, tag="pt", name="pt")
                nc.vector.tensor_scalar(pt[:], es_bc[:, ib0 * P:ib0 * P + gw],
                                        rhs_hr[blkno][:, l, 0:2].bitcast(FP32),
                                        1.0,
                                        op0=mybir.AluOpType.mult,
                                        op1=mybir.AluOpType.max)
                rhs_t = rhs_hr[blkno][:, l, ZOFF:ZOFF + F + 1]
                for a in range(ngrp):
                    nc.tensor.matmul(accs[a][:],
                                     pt[:, a * P:(a + 1) * P],
                                     rhs_t,
                                     start=(ti == 0), stop=(ti == njt - 1))

            # epilogue: attn = num/den, z2 = attn + z, softmax over F.
            # z2 is in [-14, 14] so exp is f32-safe without max-subtraction.
            # Per-bank scalar ops only where the per-block denominator
            # forces it; everything else is one wide op per pass.
            z2w = epi.tile([P, ngrp, F], FP32, tag="z2w", name="z2w")
            for a in range(ngrp):
                acc = accs[a][:]
                rden = epi.tile([P, 1], FP32, tag=f"rden{a}", name="rden")
                nc.vector.reciprocal(rden[:], acc[:, F:F + 1])
                # PSUM->SBUF stage fused with the 1/den scale; frees the bank
                nc.scalar.mul(z2w[:, a, :], acc[:, 0:F], rden[:])
            nc.vector.tensor_add(z2w[:], z2w[:], zn_sb[:, ib0:ib0 + ngrp, :])
            e2w = epi.tile([P, ngrp, F], FP32, tag="e2w", name="e2w")
            nc.scalar.activation(e2w[:], z2w[:],
                                 mybir.ActivationFunctionType.Exp)
            s6 = epi.tile([P, ngrp], FP32, tag="s6", name="s6")
            nc.vector.reduce_sum(s6[:], e2w[:], axis=mybir.AxisListType.X)
            r6 = epi.tile([P, ngrp], FP32, tag="r6", name="r6")
            nc.vector.reciprocal(r6[:], s6[:])
            o_w = epi.tile([P, ngrp, F], FP32, tag="o_w", name="o_w")
            for a in range(ngrp):
                nc.vector.tensor_scalar_mul(o_w[:, a, :], e2w[:, a, :],
                                            r6[:, a:a + 1])
            # issued from Act: the epilogue ops just above are its in-order
            # predecessors, so the SEQ blocks only briefly on o_w -- on SP
            # this issue would sit ahead of the next iteration's input DMAs
            # and stall them for the whole attention pass
            nc.scalar.dma_start(
                out_ext[ib0 * P:(ib0 + ngrp) * P, :]
                .rearrange("(a p) f -> p a f", p=P),
                o_w[:])

        if rep_b is not None:
            rep_b.__exit__(None, None, None)

    nc.compile()
    return nc


_CACHE = {}


def _get_nc(n_total=N_TOTAL, ncores=NCORES):
    key = (n_total, ncores)
    if key not in _CACHE:
        _CACHE[key] = build(n_total, ncores)
    return _CACHE[key]


def make_in_maps(x, v, g, b, att_weights, ncores=NCORES):
    n_total = x.shape[0]
    stripe = n_total // ncores
    x = np.asarray(x, np.float32)
    xT = np.ascontiguousarray(x.T.astype(ml_dtypes.bfloat16))
    v = np.ascontiguousarray(np.asarray(v, np.float32))
    vT = np.ascontiguousarray(v.T.astype(ml_dtypes.bfloat16))
    g = np.ascontiguousarray(np.asarray(g, np.float32).reshape(F, 1))
    b = np.ascontiguousarray(np.asarray(b, np.float32).reshape(F, 1))
    aw = np.ascontiguousarray(np.asarray(att_weights, np.float32).reshape(2 * F, 1))
    id128 = np.eye(P, dtype=np.float32)
    maps = []
    for c in range(ncores):
        maps.append({
            "xT": np.ascontiguousarray(xT[:, c * stripe:(c + 1) * stripe]),
            "v": v, "vT": vT, "g": g, "b": b, "aw": aw, "id128": id128,
        })
    return maps


def kernel(x, v, g, b, att_weights):
    n_total = x.shape[0]
    nc = _get_nc(n_total, NCORES)
    in_maps = make_in_maps(x, v, g, b, att_weights, NCORES)
    res = run_bass_kernel_spmd(nc, in_maps, core_ids=list(range(NCORES)))
    out = np.concatenate([res.results[c]["out"] for c in range(NCORES)], axis=0)
    return out.astype(np.float32)


# revision 30
# speedup vs baseline: 1.3880x; 1.2819x over previous
"""Distributed Bass kernel for AttnLinearEncoder (GAT-style attention encoder).

Math (reference):
    w = g * v / ||v||_row                      # weight-norm linear  [F, D]
    z = x @ w.T + b                            # [N, F]
    s = z @ a_src ; d = z @ a_dst              # [N]
    e[i, j] = relu(s_i + d_j)                  # never materialized here
    attention = softmax(e, axis=1)
    out = softmax(attention @ z + z, axis=-1)  # [N, F]

Key identity: exp(relu(u)) = max(exp(u), 1) (exp is monotonic), so the
softmax numerator P[i,j] = max(exp(s_i) * exp(d_j), 1) is a rank-1 outer
product clamped at 1 -- no transcendentals in the O(N^2) inner loop, just
one fused multiply+max per tile on the vector engine (bf16 in/out, so the
DVE runs in its 4x perf mode), feeding bf16 matmuls that accumulate both
attention@z and the softmax denominator via a ones column carried next to
z in the gathered buffer.

Sharding: rows of x are striped across 8 cores (N/8 = 1536 rows each).
Each core computes its z stripe + d stripe in bf16 (x is pre-cast to bf16
on the host; the z matmuls run at the PE's 1-cycle/row bf16 rate instead
of fp32's 4), AllGathers rank blocks of [d_f32 | z_bf16 x128 | 1 | pad]
rows (N x 132 bf16) in two halves (attention on half 1 overlaps the
gather of half 2), then computes its 1536 x N attention stripe against
the full z. d rides at the front of each row, 4-byte aligned, so rep_b
needs no separate d gather pass -- each [128,6,132] rhs tile carries its
own d column, exp'd by a tiny per-tile Act op.
"""

import numpy as np
import ml_dtypes
from contextlib import ExitStack

import concourse.bass as bass
import concourse.bacc as bacc
import concourse.mybir as mybir
import concourse.tile as tile
from concourse.bass_utils import run_bass_kernel_spmd

FP32 = mybir.dt.float32
BF16 = mybir.dt.bfloat16

N_TOTAL = 12288
D = 512
F = 128
NCORES = 8
P = 128
BW = 132            # row: d_f32(2 slots) | z(128) | ones | pad
ZOFF = 2            # z starts at slot 2; [z|1] = slots 2:131


def build(n_total=N_TOTAL, ncores=NCORES, timing_reps=0, tlsim=False,
          dummy_reads=0, rep_which="ab", a_stage=4):
    stripe = n_total // ncores          # rows per core
    nib = stripe // P                   # i-blocks of 128 own rows
    njt = n_total // P                  # j-tiles of 128 global rows
    nkc = D // P                        # k-chunks of the input dim
    nbw = min(512, stripe)              # moving free dim per z matmul
    nnb = stripe // nbw
    assert nib % 2 == 0
    nibh = nib // 2                     # i-blocks per gather half
    hst = stripe // 2                   # rows per gather half

    nc = bacc.Bacc("TRN2", target_bir_lowering=False, debug=False,
                   num_devices=1 if tlsim else ncores)

    xT = nc.dram_tensor("xT", [D, stripe], BF16, kind="ExternalInput")
    v_ext = nc.dram_tensor("v", [F, D], FP32, kind="ExternalInput")
    vT_ext = nc.dram_tensor("vT", [D, F], BF16, kind="ExternalInput")
    g_ext = nc.dram_tensor("g", [F, 1], FP32, kind="ExternalInput")
    b_ext = nc.dram_tensor("b", [F, 1], FP32, kind="ExternalInput")
    aw_ext = nc.dram_tensor("aw", [2 * F, 1], FP32, kind="ExternalInput")
    id_ext = nc.dram_tensor("id128", [P, P], FP32, kind="ExternalInput")
    out_ext = nc.dram_tensor("out", [stripe, F], FP32, kind="ExternalOutput")

    with tile.TileContext(nc) as tc, ExitStack() as ctx:
        const = ctx.enter_context(tc.tile_pool(name="const", bufs=1))
        dram = ctx.enter_context(tc.tile_pool(name="dram", bufs=1, space="DRAM"))
        psum_ctx = ExitStack()
        psum = psum_ctx.enter_context(
            tc.tile_pool(name="psum", bufs=2, space="PSUM"))
        work = ctx.enter_context(tc.tile_pool(name="work", bufs=1))

        def rep_loop(which="ab"):
            if timing_reps <= 0 or which not in rep_which:
                return None
            cm = tc.For_i(0, timing_reps, 1,
                          hint_engines=(mybir.EngineType.PE,
                                        mybir.EngineType.DVE,
                                        mybir.EngineType.Activation,
                                        mybir.EngineType.SP))
            cm.__enter__()
            return cm

        def ptile(shape, dt=FP32):
            # transient PSUM tiles share the "tmp" tag -> 2 rotating slots
            return psum.tile(shape, dt, tag="tmp", name="ptmp")

        # rank block layout (bf16 elems): hst rows of BW = [d|z|1|pad]
        zc_loc = [dram.tile([hst * BW], BF16, name=f"zc_loc{h}") for h in (0, 1)]
        zc_full = [dram.tile([ncores * hst * BW], BF16, addr_space="Shared",
                             name=f"zc_full{h}") for h in (0, 1)]

        def blk(buf, base):        # [hst, BW] rows of one rank block
            return buf[base:base + hst * BW].rearrange("(i w) -> i w", w=BW)

        # ---- constants -------------------------------------------------
        v_sb = const.tile([P, D], FP32)
        vT_sb = const.tile([P, nkc, F], BF16)
        g_sb = const.tile([P, 1], FP32)
        b_sb = const.tile([P, 1], FP32)
        asad = const.tile([P, 2], BF16)
        asad32 = const.tile([P, 2], FP32)
        ident = const.tile([P, P], FP32)
        ones_row = const.tile([1, P], BF16)
        nc.vector.memset(ones_row[:], 1.0)
        nc.gpsimd.dma_start(v_sb[:], v_ext[:])
        nc.gpsimd.dma_start(vT_sb[:], vT_ext.ap().rearrange("(c p) f -> p c f", p=P))
        nc.gpsimd.dma_start(g_sb[:], g_ext[:])
        nc.gpsimd.dma_start(b_sb[:], b_ext[:])
        nc.gpsimd.dma_start(asad32[:, 0:1], aw_ext[0:F, :])
        nc.gpsimd.dma_start(asad32[:, 1:2], aw_ext[F:2 * F, :])
        nc.gpsimd.dma_start(ident[:], id_ext[:])
        nc.vector.tensor_copy(asad[:], asad32[:])

        xc = [work.tile([P, stripe], BF16, name=f"xc{c}") for c in range(nkc)]
        probe_fill = []     # timing-probe stages skip writers; fill before loop
        rep_a = rep_loop("a")
        xT_v = xT.ap().rearrange("(c p) i -> c p i", p=P)
        if a_stage >= 1:
            for c in range(nkc):
                # split the input stream across both HWDGE queues
                eng = nc.sync if c % 2 == 0 else nc.scalar
                eng.dma_start(xc[c][:], xT_v[c])

        # ---- weight prep: scale = g / ||v||_row ------------------------
        # The scale never touches the weights: z = (x @ v.T) * scale + b is
        # applied per-partition at the PSUM eviction, so the z matmuls start
        # as soon as vT and the first x chunk land.
        v2 = work.tile([P, D], FP32)
        nrm2 = work.tile([P, 1], FP32)
        nrm = work.tile([P, 1], FP32)
        rinv = work.tile([P, 1], FP32)
        scale_w = work.tile([P, 1], FP32)
        if a_stage >= 2:
            nc.vector.tensor_mul(v2[:], v_sb[:], v_sb[:])
            nc.vector.reduce_sum(nrm2[:], v2[:], axis=mybir.AxisListType.X)
            nc.scalar.sqrt(nrm[:], nrm2[:])
            nc.vector.reciprocal(rinv[:], nrm[:])
            nc.vector.tensor_mul(scale_w[:], rinv[:], g_sb[:])

        # ---- z stripe (transposed) + exp(s)/exp(d) from zT -------------
        # s = z @ a_src, d = z @ a_dst (scale/bias already folded into z)
        zT_sb = work.tile([P, stripe], FP32)
        zTb_sb = work.tile([P, stripe], BF16)
        esd_sb = work.tile([2, stripe], FP32)
        esb_sb = work.tile([1, stripe], BF16)
        for nb in range(nnb if a_stage >= 2 else 0):
            sl = slice(nb * nbw, (nb + 1) * nbw)
            zt_ps = ptile([P, nbw])
            for c in range(nkc):
                nc.tensor.matmul(zt_ps[:], vT_sb[:, c, :], xc[c][:, sl],
                                 start=(c == 0), stop=(c == nkc - 1))
            nc.scalar.activation(zT_sb[:, sl], zt_ps[:],
                                 mybir.ActivationFunctionType.Identity,
                                 bias=b_sb[:], scale=scale_w[:])
            nc.vector.tensor_copy(zTb_sb[:, sl], zT_sb[:, sl])
            if a_stage < 3:
                continue
            sd_ps = ptile([2, nbw])
            nc.tensor.matmul(sd_ps[:], asad[:], zTb_sb[:, sl],
                             start=True, stop=True)
            # exp both rows straight out of PSUM: [exp(s); exp(d)]
            nc.scalar.activation(esd_sb[:, sl], sd_ps[:],
                                 mybir.ActivationFunctionType.Exp)
            nc.vector.tensor_copy(esb_sb[:, sl], esd_sb[0:1, sl])

        # z natural layout: f32 for +z / output, bf16 (+ones col) for gather
        zn_sb = work.tile([P, nib, F], FP32)
        znb_sb = work.tile([P, nib, F + 1], BF16)
        nc.vector.memset(znb_sb[:, :, F:F + 1], 1.0)
        for ib in range(nib if a_stage >= 4 else 0):
            zn_ps = ptile([P, P])
            nc.tensor.transpose(zn_ps[:], zT_sb[:, ib * P:(ib + 1) * P], ident[:])
            nc.scalar.copy(zn_sb[:, ib, :], zn_ps[:])
            nc.vector.tensor_copy(znb_sb[:, ib, 0:F], zn_sb[:, ib, :])
        for h in ((0, 1) if a_stage >= 4 else ()):
            # one [z|1] row-block write per half, one d column write per half
            eng = nc.sync if h == 0 else nc.scalar
            eng.dma_start(
                blk(zc_loc[h], 0)[:, ZOFF:ZOFF + F + 1]
                .rearrange("(q p) w -> p q w", p=P),
                znb_sb[:, h * nibh:(h + 1) * nibh, :])
            # exp(d) rides at the front of each row, 4-byte aligned
            eng.dma_start(
                blk(zc_loc[h], 0)[:, 0:2].bitcast(FP32),
                esd_sb[1:2, h * hst:(h + 1) * hst])

        # Es[i] = exp(s_i) broadcast over partitions, bf16 [128, stripe]
        # (depends only on local sd, so it runs under the all-gather)
        es_bc = work.tile([P, stripe], BF16)
        for nb in range(nnb if a_stage >= 4 else 0):
            sl = slice(nb * nbw, (nb + 1) * nbw)
            es_ps = ptile([P, nbw])
            nc.tensor.matmul(es_ps[:], ones_row[:], esb_sb[:, sl],
                             start=True, stop=True)
            nc.vector.tensor_copy(es_bc[:, sl], es_ps[:])

        if rep_a is not None:
            rep_a.__exit__(None, None, None)
        if a_stage < 4:
            for t in (es_bc, zn_sb, znb_sb, esd_sb, esb_sb, zTb_sb, zT_sb,
                      v2, nrm2, nrm, rinv, scale_w):
                nc.vector.memset(t[:], 1.0)
            for c in range(nkc):
                nc.vector.memset(xc[c][:], 1.0)

        # ---- all-gather [d | z | 1], two halves ------------------------
        for h in (0, 1):
            if tlsim:
                nc.gpsimd.dma_start(zc_full[h][0:hst * BW], zc_loc[h][:])
            else:
                nc.gpsimd.collective_compute(
                    "AllGather",
                    mybir.AluOpType.bypass,
                    ins=[zc_loc[h][:].opt()],
                    outs=[zc_full[h][:].opt()],
                    replica_groups=[list(range(ncores))],
                )

        # j-tile t -> (half, row block) in the gathered buffers
        def t_loc(t):
            r, l = divmod(t, nib)
            h, lb = divmod(l, nibh)
            return h, (r * nibh + lb)

        torder = sorted(range(njt), key=lambda t: t_loc(t))

        rep_b = rep_loop("b")
        # ---- post-gather prep -----------------------------------------
        # gathered rows land in SBUF in gather order, one tile+DMA per
        # (half, rank-pair) so the attention can start after the first
        # block; each tile carries exp(d)(f32) | z | 1 per row, so there
        # is NO prep compute at all -- the pt build reads its per-row
        # exp(d) scalar straight out of the rhs tile via a strided bitcast
        # slice. (DMA issues cost ~0.7us of sequencer time each, so fewer,
        # larger transfers win.)
        rpb = 2                          # ranks per rhs block
        nrb = ncores // rpb              # rhs blocks per half
        bq = rpb * nibh                  # j-tiles per rhs block
        rhs_hr = [work.tile([P, bq, BW], BF16, name=f"rhs{h}_{r}")
                  for h in (0, 1) for r in range(nrb)]
        for h in (0, 1):
            for r in range(nrb):
                eng = nc.sync if r % 2 == 0 else nc.scalar
                base = r * rpb * hst * BW
                src = (zc_full[h][base:base + rpb * hst * BW]
                       .rearrange("(q p w) -> p q w", p=P, w=BW))
                eng.dma_start(rhs_hr[h * nrb + r][:], src)
        if dummy_reads:
            # timing probe only: re-read the gathered payload into a scratch
            # tile nothing consumes, to measure the DRAM bandwidth cost
            scratch = [work.tile([P, bq, BW], BF16, name=f"scr{h}_{r}")
                       for h in (0, 1) for r in range(nrb)]
            for h in (0, 1):
                for r in range(nrb):
                    eng = nc.sync if r % 2 == 0 else nc.scalar
                    base = r * rpb * hst * BW
                    src = (zc_full[h][base:base + rpb * hst * BW]
                           .rearrange("(q p w) -> p q w", p=P, w=BW))
                    eng.dma_start(scratch[h * nrb + r][:], src)

        # ---- attention stripe: accumulate P.T @ [z|1] over all j ------
        # One PSUM bank per i-block accumulator; the tmp psum pool is
        # closed here so all 8 banks are available: passes of 8 then 4
        # (shorter final epilogue tail).
        psum_ctx.close()
        apsum = ctx.enter_context(tc.tile_pool(name="apsum", bufs=1, space="PSUM"))
        ptp = ctx.enter_context(tc.tile_pool(name="ptp", bufs=4))
        epi = ctx.enter_context(tc.tile_pool(name="epi", bufs=4))
        ib_group = 8
        for ib0 in range(0, nib, ib_group):
            ngrp = min(ib_group, nib - ib0)
            gw = ngrp * P
            accs = [apsum.tile([P, F + 1], FP32, name=f"acc{a}", tag=f"acc{a}")
                    for a in range(ngrp)]
            for ti, t in enumerate(torder):
                blkno, l = divmod(ti, bq)
                pt = ptp.tile([P, gw], BF16, tag="pt", name="pt")
                nc.vector.tensor_scalar(pt[:], es_bc[:, ib0 * P:ib0 * P + gw],
                                        rhs_hr[blkno][:, l, 0:2].bitcast(FP32),
                                        1.0,
                                        op0=mybir.AluOpType.mult,
                                        op1=mybir.AluOpType.max)
                rhs_t = rhs_hr[blkno][:, l, ZOFF:ZOFF + F + 1]
                for a in range(ngrp):
                    nc.tensor.matmul(accs[a][:],
                                     pt[:, a * P:(a + 1) * P],
                                     rhs_t,
                                     start=(ti == 0), stop=(ti == njt - 1))

            # epilogue: attn = num/den, z2 = attn + z, softmax over F.
            # z2 is in [-14, 14] so exp is f32-safe without max-subtraction.
            # Per-bank scalar ops only where the per-block denominator
            # forces it; everything else is one wide op per pass.
            z2w = epi.tile([P, ngrp, F], FP32, tag="z2w", name="z2w")
            for a in range(ngrp):
                acc = accs[a][:]
                rden = epi.tile([P, 1], FP32, tag=f"rden{a}", name="rden")
                nc.vector.reciprocal(rden[:], acc[:, F:F + 1])
                # PSUM->SBUF stage fused with the 1/den scale; frees the bank
                nc.scalar.mul(z2w[:, a, :], acc[:, 0:F], rden[:])
            nc.vector.tensor_add(z2w[:], z2w[:], zn_sb[:, ib0:ib0 + ngrp, :])
            e2w = epi.tile([P, ngrp, F], FP32, tag="e2w", name="e2w")
            nc.scalar.activation(e2w[:], z2w[:],
                                 mybir.ActivationFunctionType.Exp)
            s6 = epi.tile([P, ngrp], FP32, tag="s6", name="s6")
            nc.vector.reduce_sum(s6[:], e2w[:], axis=mybir.AxisListType.X)
            r6 = epi.tile([P, ngrp], FP32, tag="r6", name="r6")
            nc.vector.reciprocal(r6[:], s6[:])
            o_w = epi.tile([P, ngrp, F], FP32, tag="o_w", name="o_w")
            for a in range(ngrp):
                nc.vector.tensor_scalar_mul(o_w[:, a, :], e2w[:, a, :],
                                            r6[:, a:a + 1])
            # issued from Act: the epilogue ops just above are its in-order
            # predecessors, so the SEQ blocks only briefly on o_w -- on SP
            # this issue would sit ahead of the next iteration's input DMAs
            # and stall them for the whole attention pass
            nc.scalar.dma_start(
                out_ext[ib0 * P:(ib0 + ngrp) * P, :]
                .rearrange("(a p) f -> p a f", p=P),
                o_w[:])

        if rep_b is not None:
            rep_b.__exit__(None, None, None)

    nc.compile()
    return nc


_CACHE = {}


def _get_nc(n_total=N_TOTAL, ncores=NCORES):
    key = (n_total, ncores)
    if key not in _CACHE:
        _CACHE[key] = build(n_total, ncores)
    return _CACHE[key]


def make_in_maps(x, v, g, b, att_weights, ncores=NCORES):
    n_total = x.shape[0]
    stripe = n_total // ncores
    x = np.asarray(x, np.float32)
    xT = np.ascontiguousarray(x.T.astype(ml_dtypes.bfloat16))
    v = np.ascontiguousarray(np.asarray(v, np.float32))
    vT = np.ascontiguousarray(v.T.astype(ml_dtypes.bfloat16))
    g = np.ascontiguousarray(np.asarray(g, np.float32).reshape(F, 1))
    b = np.ascontiguousarray(np.asarray(b, np.float32).reshape(F, 1))
    aw = np.ascontiguousarray(np.asarray(att_weights, np.float32).reshape(2 * F, 1))
    id128 = np.eye(P, dtype=np.float32)
    maps = []
    for c in range(ncores):
        maps.append({
            "xT": np.ascontiguousarray(xT[:, c * stripe:(c + 1) * stripe]),
            "v": v, "vT": vT, "g": g, "b": b, "aw": aw, "id128": id128,
        })
    return maps


def kernel(x, v, g, b, att_weights):
    n_total = x.shape[0]
    nc = _get_nc(n_total, NCORES)
    in_maps = make_in_maps(x, v, g, b, att_weights, NCORES)
    res = run_bass_kernel_spmd(nc, in_maps, core_ids=list(range(NCORES)))
    out = np.concatenate([res.results[c]["out"] for c in range(NCORES)], axis=0)
    return out.astype(np.float32)


# revision 38
# speedup vs baseline: 1.4714x; 1.0601x over previous
"""Distributed Bass kernel for AttnLinearEncoder (GAT-style attention encoder).

Math (reference):
    w = g * v / ||v||_row                      # weight-norm linear  [F, D]
    z = x @ w.T + b                            # [N, F]
    s = z @ a_src ; d = z @ a_dst              # [N]
    e[i, j] = relu(s_i + d_j)                  # never materialized here
    attention = softmax(e, axis=1)
    out = softmax(attention @ z + z, axis=-1)  # [N, F]

Key identity: exp(relu(u)) = max(exp(u), 1) (exp is monotonic), so the
softmax numerator P[i,j] = max(exp(s_i) * exp(d_j), 1) is a rank-1 outer
product clamped at 1 -- no transcendentals in the O(N^2) inner loop, just
one fused multiply+max per tile on the vector engine (bf16 in/out, so the
DVE runs in its 4x perf mode), feeding bf16 matmuls that accumulate both
attention@z and the softmax denominator via a ones column carried next to
z in the gathered buffer.

Sharding: rows of x are striped across 8 cores (N/8 = 1536 rows each).
Each core computes its z stripe + d stripe in bf16 (x is pre-cast to bf16
on the host; the z matmuls run at the PE's 1-cycle/row bf16 rate instead
of fp32's 4), AllGathers rank blocks of [d_f32 | z_bf16 x128 | 1 | pad]
rows (N x 132 bf16) in two halves (attention on half 1 overlaps the
gather of half 2), then computes its 1536 x N attention stripe against
the full z. d rides at the front of each row, 4-byte aligned, so rep_b
needs no separate d gather pass -- each [128,6,132] rhs tile carries its
own d column, exp'd by a tiny per-tile Act op.
"""

import numpy as np
import ml_dtypes
from contextlib import ExitStack

import concourse.bass as bass
import concourse.bacc as bacc
import concourse.mybir as mybir
import concourse.tile as tile
from concourse.bass_utils import run_bass_kernel_spmd

FP32 = mybir.dt.float32
BF16 = mybir.dt.bfloat16

N_TOTAL = 12288
D = 512
F = 128
NCORES = 8
P = 128
BW = 132            # row: d_f32(2 slots) | z(128) | ones | pad
ZOFF = 2            # z starts at slot 2; [z|1] = slots 2:131


def build(n_total=N_TOTAL, ncores=NCORES, timing_reps=0, tlsim=False,
          dummy_reads=0, rep_which="ab", a_stage=4, pt_bufs=6, ib_group=8):
    stripe = n_total // ncores          # rows per core
    nib = stripe // P                   # i-blocks of 128 own rows
    njt = n_total // P                  # j-tiles of 128 global rows
    nkc = D // P                        # k-chunks of the input dim
    nbw = min(512, stripe)              # moving free dim per z matmul
    nnb = stripe // nbw
    assert nib % 2 == 0
    nibh = nib // 2                     # i-blocks per gather half
    hst = stripe // 2                   # rows per gather half

    nc = bacc.Bacc("TRN2", target_bir_lowering=False, debug=False,
                   num_devices=1 if tlsim else ncores)

    xT = nc.dram_tensor("xT", [D, stripe], BF16, kind="ExternalInput")
    v_ext = nc.dram_tensor("v", [F, D], FP32, kind="ExternalInput")
    vT_ext = nc.dram_tensor("vT", [D, F], BF16, kind="ExternalInput")
    g_ext = nc.dram_tensor("g", [F, 1], FP32, kind="ExternalInput")
    b_ext = nc.dram_tensor("b", [F, 1], FP32, kind="ExternalInput")
    aw_ext = nc.dram_tensor("aw", [2 * F, 1], FP32, kind="ExternalInput")
    id_ext = nc.dram_tensor("id128", [P, P], FP32, kind="ExternalInput")
    out_ext = nc.dram_tensor("out", [stripe, F], FP32, kind="ExternalOutput")

    with tile.TileContext(nc) as tc, ExitStack() as ctx:
        const = ctx.enter_context(tc.tile_pool(name="const", bufs=1))
        dram = ctx.enter_context(tc.tile_pool(name="dram", bufs=1, space="DRAM"))
        psum_ctx = ExitStack()
        psum = psum_ctx.enter_context(
            tc.tile_pool(name="psum", bufs=2, space="PSUM"))
        work = ctx.enter_context(tc.tile_pool(name="work", bufs=1))

        def rep_loop(which="ab"):
            if timing_reps <= 0 or which not in rep_which:
                return None
            cm = tc.For_i(0, timing_reps, 1,
                          hint_engines=(mybir.EngineType.PE,
                                        mybir.EngineType.DVE,
                                        mybir.EngineType.Activation,
                                        mybir.EngineType.SP))
            cm.__enter__()
            return cm

        def ptile(shape, dt=FP32):
            # transient PSUM tiles share the "tmp" tag -> 2 rotating slots
            return psum.tile(shape, dt, tag="tmp", name="ptmp")

        # rank block layout (bf16 elems): hst rows of BW = [d|z|1|pad]
        zc_loc = [dram.tile([hst * BW], BF16, name=f"zc_loc{h}") for h in (0, 1)]
        zc_full = [dram.tile([ncores * hst * BW], BF16, addr_space="Shared",
                             name=f"zc_full{h}") for h in (0, 1)]

        def blk(buf, base):        # [hst, BW] rows of one rank block
            return buf[base:base + hst * BW].rearrange("(i w) -> i w", w=BW)

        # ---- constants -------------------------------------------------
        v_sb = const.tile([P, D], FP32)
        vT_sb = const.tile([P, nkc, F], BF16)
        g_sb = const.tile([P, 1], FP32)
        b_sb = const.tile([P, 1], FP32)
        asad32 = const.tile([P, 2], FP32)
        ident = const.tile([P, P], FP32)
        ones_row = const.tile([1, P], BF16)
        nc.vector.memset(ones_row[:], 1.0)
        nc.gpsimd.dma_start(v_sb[:], v_ext[:])
        nc.gpsimd.dma_start(vT_sb[:], vT_ext.ap().rearrange("(c p) f -> p c f", p=P))
        nc.gpsimd.dma_start(g_sb[:], g_ext[:])
        nc.gpsimd.dma_start(b_sb[:], b_ext[:])
        nc.gpsimd.dma_start(asad32[:, 0:1], aw_ext[0:F, :])
        nc.gpsimd.dma_start(asad32[:, 1:2], aw_ext[F:2 * F, :])
        nc.gpsimd.dma_start(ident[:], id_ext[:])
        # ---- one-time weight prep: scale = g / ||v||_row ---------------
        # The scale never touches the weights: z = (x @ v.T) * scale + b is
        # applied per-partition at the PSUM eviction. s/d come straight
        # from x: [s; d] = x @ wa + b@asad, with wa = v.T @ (scale * asad)
        # [D, 2] -- so the sd matmuls run off the same xc chunks as z
        # instead of waiting for the z eviction.
        v2 = work.tile([P, D], FP32)
        nrm2 = work.tile([P, 1], FP32)
        nrm = work.tile([P, 1], FP32)
        rinv = work.tile([P, 1], FP32)
        scale_w = work.tile([P, 1], FP32)
        nc.vector.tensor_mul(v2[:], v_sb[:], v_sb[:])
        nc.vector.reduce_sum(nrm2[:], v2[:], axis=mybir.AxisListType.X)
        nc.scalar.sqrt(nrm[:], nrm2[:])
        nc.vector.reciprocal(rinv[:], nrm[:])
        nc.vector.tensor_mul(scale_w[:], rinv[:], g_sb[:])
        sa32 = work.tile([P, 2], FP32)
        nc.vector.tensor_scalar_mul(sa32[:], asad32[:], scale_w[:])
        wa_sb = work.tile([P, nkc, 2], BF16)
        for c in range(nkc):
            wa_ps = ptile([P, 2])
            nc.tensor.matmul(wa_ps[:], v_sb[:, c * P:(c + 1) * P], sa32[:],
                             start=True, stop=True)
            nc.scalar.copy(wa_sb[:, c, :], wa_ps[:])
        bias_sd = work.tile([2, 1], FP32)
        bias_ps = ptile([2, 1])
        nc.tensor.matmul(bias_ps[:], asad32[:], b_sb[:], start=True, stop=True)
        nc.scalar.copy(bias_sd[:], bias_ps[:])

        xc = [work.tile([P, stripe], BF16, name=f"xc{c}") for c in range(nkc)]
        rep_a = rep_loop("a")
        xT_v = xT.ap().rearrange("(c p) i -> c p i", p=P)
        if a_stage >= 1:
            for c in range(nkc):
                # split the input stream across both HWDGE queues
                eng = nc.sync if c % 2 == 0 else nc.scalar
                eng.dma_start(xc[c][:], xT_v[c])

        # ---- z stripe (transposed) + exp(s)/exp(d) straight from x -----
        zT_sb = work.tile([P, stripe], FP32)
        esd_sb = work.tile([2, stripe], FP32)
        esb_sb = work.tile([1, stripe], BF16)
        for nb in range(nnb if a_stage >= 2 else 0):
            sl = slice(nb * nbw, (nb + 1) * nbw)
            zt_ps = ptile([P, nbw])
            for c in range(nkc):
                nc.tensor.matmul(zt_ps[:], vT_sb[:, c, :], xc[c][:, sl],
                                 start=(c == 0), stop=(c == nkc - 1))
            nc.scalar.activation(zT_sb[:, sl], zt_ps[:],
                                 mybir.ActivationFunctionType.Identity,
                                 bias=b_sb[:], scale=scale_w[:])
            if a_stage < 3:
                continue
            sd_ps = ptile([2, nbw])
            for c in range(nkc):
                nc.tensor.matmul(sd_ps[:], wa_sb[:, c, :], xc[c][:, sl],
                                 start=(c == 0), stop=(c == nkc - 1))
            # exp both rows straight out of PSUM: [exp(s+bs); exp(d+bd)]
            nc.scalar.activation(esd_sb[:, sl], sd_ps[:],
                                 mybir.ActivationFunctionType.Exp,
                                 bias=bias_sd[:])
            nc.vector.tensor_copy(esb_sb[:, sl], esd_sb[0:1, sl])

        # z natural layout: f32 for +z / output, bf16 (+ones col) for gather
        zn_sb = work.tile([P, nib, F], FP32)
        znb_sb = work.tile([P, nib, F + 1], BF16)
        nc.vector.memset(znb_sb[:, :, F:F + 1], 1.0)
        for ib in range(nib if a_stage >= 4 else 0):
            zn_ps = ptile([P, P])
            nc.tensor.transpose(zn_ps[:], zT_sb[:, ib * P:(ib + 1) * P], ident[:])
            nc.scalar.copy(zn_sb[:, ib, :], zn_ps[:])
            nc.vector.tensor_copy(znb_sb[:, ib, 0:F], zn_sb[:, ib, :])
        for h in ((0, 1) if a_stage >= 4 else ()):
            # one [z|1] row-block write per half, one d column write per half
            eng = nc.sync if h == 0 else nc.scalar
            eng.dma_start(
                blk(zc_loc[h], 0)[:, ZOFF:ZOFF + F + 1]
                .rearrange("(q p) w -> p q w", p=P),
                znb_sb[:, h * nibh:(h + 1) * nibh, :])
            # exp(d) rides at the front of each row, 4-byte aligned
            eng.dma_start(
                blk(zc_loc[h], 0)[:, 0:2].bitcast(FP32),
                esd_sb[1:2, h * hst:(h + 1) * hst])

        # Es[i] = exp(s_i) broadcast over partitions, bf16 [128, stripe]
        # (depends only on local sd, so it runs under the all-gather)
        es_bc = work.tile([P, stripe], BF16)
        for nb in range(nnb if a_stage >= 4 else 0):
            sl = slice(nb * nbw, (nb + 1) * nbw)
            es_ps = ptile([P, nbw])
            nc.tensor.matmul(es_ps[:], ones_row[:], esb_sb[:, sl],
                             start=True, stop=True)
            nc.vector.tensor_copy(es_bc[:, sl], es_ps[:])

        if rep_a is not None:
            rep_a.__exit__(None, None, None)
        if a_stage < 4:
            for t in (es_bc, zn_sb, znb_sb, esd_sb, esb_sb, zT_sb):
                nc.vector.memset(t[:], 1.0)
            for c in range(nkc):
                nc.vector.memset(xc[c][:], 1.0)

        # ---- all-gather [d | z | 1], two halves ------------------------
        for h in (0, 1):
            if tlsim:
                nc.gpsimd.dma_start(zc_full[h][0:hst * BW], zc_loc[h][:])
            else:
                nc.gpsimd.collective_compute(
                    "AllGather",
                    mybir.AluOpType.bypass,
                    ins=[zc_loc[h][:].opt()],
                    outs=[zc_full[h][:].opt()],
                    replica_groups=[list(range(ncores))],
                )

        # j-tile t -> (half, row block) in the gathered buffers
        def t_loc(t):
            r, l = divmod(t, nib)
            h, lb = divmod(l, nibh)
            return h, (r * nibh + lb)

        torder = sorted(range(njt), key=lambda t: t_loc(t))

        rep_b = rep_loop("b")
        # ---- post-gather prep -----------------------------------------
        # gathered rows land in SBUF in gather order, one tile+DMA per
        # (half, rank-pair) so the attention can start after the first
        # block; each tile carries exp(d)(f32) | z | 1 per row, so there
        # is NO prep compute at all -- the pt build reads its per-row
        # exp(d) scalar straight out of the rhs tile via a strided bitcast
        # slice. (DMA issues cost ~0.7us of sequencer time each, so fewer,
        # larger transfers win.)
        rpb = 2                          # ranks per rhs block
        nrb = ncores // rpb              # rhs blocks per half
        bq = rpb * nibh                  # j-tiles per rhs block
        rhs_hr = [work.tile([P, bq, BW], BF16, name=f"rhs{h}_{r}")
                  for h in (0, 1) for r in range(nrb)]
        for h in (0, 1):
            for r in range(nrb):
                eng = nc.sync if r % 2 == 0 else nc.scalar
                base = r * rpb * hst * BW
                src = (zc_full[h][base:base + rpb * hst * BW]
                       .rearrange("(q p w) -> p q w", p=P, w=BW))
                eng.dma_start(rhs_hr[h * nrb + r][:], src)
        if dummy_reads:
            # timing probe only: re-read the gathered payload into a scratch
            # tile nothing consumes, to measure the DRAM bandwidth cost
            scratch = [work.tile([P, bq, BW], BF16, name=f"scr{h}_{r}")
                       for h in (0, 1) for r in range(nrb)]
            for h in (0, 1):
                for r in range(nrb):
                    eng = nc.sync if r % 2 == 0 else nc.scalar
                    base = r * rpb * hst * BW
                    src = (zc_full[h][base:base + rpb * hst * BW]
                           .rearrange("(q p w) -> p q w", p=P, w=BW))
                    eng.dma_start(scratch[h * nrb + r][:], src)

        # ---- attention stripe: accumulate P.T @ [z|1] over all j ------
        # One PSUM bank per i-block accumulator; the tmp psum pool is
        # closed here so all 8 banks are available: passes of 8 then 4
        # (shorter final epilogue tail).
        psum_ctx.close()
        apsum = ctx.enter_context(tc.tile_pool(name="apsum", bufs=1, space="PSUM"))
        ptp = ctx.enter_context(tc.tile_pool(name="ptp", bufs=pt_bufs))
        epi = ctx.enter_context(tc.tile_pool(name="epi", bufs=4))
        for ib0 in range(0, nib, ib_group):
            ngrp = min(ib_group, nib - ib0)
            gw = ngrp * P
            accs = [apsum.tile([P, F + 1], FP32, name=f"acc{a}", tag=f"acc{a}")
                    for a in range(ngrp)]
            for ti, t in enumerate(torder):
                blkno, l = divmod(ti, bq)
                pt = ptp.tile([P, gw], BF16, tag="pt", name="pt")
                nc.vector.tensor_scalar(pt[:], es_bc[:, ib0 * P:ib0 * P + gw],
                                        rhs_hr[blkno][:, l, 0:2].bitcast(FP32),
                                        1.0,
                                        op0=mybir.AluOpType.mult,
                                        op1=mybir.AluOpType.max)
                rhs_t = rhs_hr[blkno][:, l, ZOFF:ZOFF + F + 1]
                for a in range(ngrp):
                    nc.tensor.matmul(accs[a][:],
                                     pt[:, a * P:(a + 1) * P],
                                     rhs_t,
                                     start=(ti == 0), stop=(ti == njt - 1))

            # epilogue: attn = num/den, z2 = attn + z, softmax over F.
            # z2 is in [-14, 14] so exp is f32-safe without max-subtraction.
            # Per-bank scalar ops only where the per-block denominator
            # forces it; everything else is one wide op per pass.
            z2w = epi.tile([P, ngrp, F], FP32, tag="z2w", name="z2w")
            for a in range(ngrp):
                acc = accs[a][:]
                rden = epi.tile([P, 1], FP32, tag=f"rden{a}", name="rden")
                nc.vector.reciprocal(rden[:], acc[:, F:F + 1])
                # PSUM->SBUF stage fused with the 1/den scale; frees the bank
                nc.scalar.mul(z2w[:, a, :], acc[:, 0:F], rden[:])
            nc.vector.tensor_add(z2w[:], z2w[:], zn_sb[:, ib0:ib0 + ngrp, :])
            e2w = epi.tile([P, ngrp, F], FP32, tag="e2w", name="e2w")
            nc.scalar.activation(e2w[:], z2w[:],
                                 mybir.ActivationFunctionType.Exp)
            s6 = epi.tile([P, ngrp], FP32, tag="s6", name="s6")
            nc.vector.reduce_sum(s6[:], e2w[:], axis=mybir.AxisListType.X)
            r6 = epi.tile([P, ngrp], FP32, tag="r6", name="r6")
            nc.vector.reciprocal(r6[:], s6[:])
            o_w = epi.tile([P, ngrp, F], FP32, tag="o_w", name="o_w")
            for a in range(ngrp):
                nc.vector.tensor_scalar_mul(o_w[:, a, :], e2w[:, a, :],
                                            r6[:, a:a + 1])
            # issued from Act: the epilogue ops just above are its in-order
            # predecessors, so the SEQ blocks only briefly on o_w -- on SP
            # this issue would sit ahead of the next iteration's input DMAs
            # and stall them for the whole attention pass
            nc.scalar.dma_start(
                out_ext[ib0 * P:(ib0 + ngrp) * P, :]
                .rearrange("(a p) f -> p a f", p=P),
                o_w[:])

        if rep_b is not None:
            rep_b.__exit__(None, None, None)

    nc.compile()
    return nc


_CACHE = {}


def _get_nc(n_total=N_TOTAL, ncores=NCORES):
    key = (n_total, ncores)
    if key not in _CACHE:
        _CACHE[key] = build(n_total, ncores)
    return _CACHE[key]


def make_in_maps(x, v, g, b, att_weights, ncores=NCORES):
    n_total = x.shape[0]
    stripe = n_total // ncores
    x = np.asarray(x, np.float32)
    xT = np.ascontiguousarray(x.T.astype(ml_dtypes.bfloat16))
    v = np.ascontiguousarray(np.asarray(v, np.float32))
    vT = np.ascontiguousarray(v.T.astype(ml_dtypes.bfloat16))
    g = np.ascontiguousarray(np.asarray(g, np.float32).reshape(F, 1))
    b = np.ascontiguousarray(np.asarray(b, np.float32).reshape(F, 1))
    aw = np.ascontiguousarray(np.asarray(att_weights, np.float32).reshape(2 * F, 1))
    id128 = np.eye(P, dtype=np.float32)
    maps = []
    for c in range(ncores):
        maps.append({
            "xT": np.ascontiguousarray(xT[:, c * stripe:(c + 1) * stripe]),
            "v": v, "vT": vT, "g": g, "b": b, "aw": aw, "id128": id128,
        })
    return maps


def kernel(x, v, g, b, att_weights):
    n_total = x.shape[0]
    nc = _get_nc(n_total, NCORES)
    in_maps = make_in_maps(x, v, g, b, att_weights, NCORES)
    res = run_bass_kernel_spmd(nc, in_maps, core_ids=list(range(NCORES)))
    out = np.concatenate([res.results[c]["out"] for c in range(NCORES)], axis=0)
    return out.astype(np.float32)
